# revision 26
# baseline (speedup 1.0000x reference)
"""BondPredictor (GNN message passing) Trainium2 kernel — 8 NeuronCores.

Strategy
--------
reference:
    node_emb = (x @ Wa + ba) + (pos @ Wp + bp)            # [N,128]
    e = concat([node_emb[src], node_emb[dst], dist], -1)  # [E,257]
    h = silu(e @ W1 + b1); h = silu(h @ W2 + b2); out = h @ W3 + b3

Key algebraic fold: because node_emb is linear in [x, pos, 1],

    e @ W1 + b1 = [x_s, p_s, x_d, p_d, dist] @ W1cat + b1'

with W1cat = [Wa@W1a; Wp@W1a; Wa@W1b; Wp@W1b; w1c]  (39 x 128) and
b1' = (ba+bp)@(W1a+W1b) + b1.  So the device kernel needs NO gathers at
all: the host lays out a dense per-edge feature stream of 39 fp16 rows
(src features, dst features, distance) in edge order, and each core runs
a pure 3-matmul MLP pipeline over its contiguous slice of edges:

    MM1 [39->128] -> silu -> MM2 [128->128] -> silu -> MM3 [128->4]

(A previous revision gathered 256B node_emb rows per edge with
dma_gather; the NTFF trace showed the Pool engine spending 3.5ms/core
generating DMA descriptors at ~8ns each — per-edge descriptors are the
real bottleneck on TRN2, so they were eliminated entirely.)

Sharding: contiguous edge ranges, 200000 edges per core.  b3 is added on
the host during output reassembly (it is a constant [4] vector).
"""

import sys

for _p in ("/opt/trn_rl_repo",):
    if _p not in sys.path:
        sys.path.insert(0, _p)

import numpy as np

import concourse.bass as bass
import concourse.bacc as bacc
import concourse.mybir as mybir
import concourse.tile as tile
from concourse import bass_utils

F16 = mybir.dt.float16
F32 = mybir.dt.float32

# ---------------------------------------------------------------- config
N_NODES = 100000
N_EDGES = 1600000
ATOM = 16
POSD = 3
HID = 128
NOUT = 4
N_CORES = 8

RVEC = 2 * (ATOM + POSD) + 1    # 39 stream rows per edge
EC = N_EDGES // N_CORES         # 200000 edges per core
TILE = 1024                     # edges per compute tile
MMR = 512                       # matmul free-dim region (one PSUM bank)
TILES_PER_LOAD = 4              # tiles per input DMA
LOADW = TILE * TILES_PER_LOAD   # 4096 edges per input DMA
NTILES = (EC + TILE - 1) // TILE            # 196
NLOADS = (NTILES + TILES_PER_LOAD - 1) // TILES_PER_LOAD  # 49
ESLOT = NLOADS * LOADW          # 200704 padded edge slots per core
OUTW = (TILE // 128) * NOUT     # 32 output cols per tile (FWL layout)
GRP = 7                         # tiles staged per output DMA
NGRP = NTILES // GRP            # 28

_CACHE = {}


# ---------------------------------------------------------------- program
def _build_program():
    nc = bacc.Bacc("TRN2", target_bir_lowering=False, debug=False,
                   num_devices=N_CORES)

    dt = nc.dram_tensor
    stream = dt("stream", [NLOADS, RVEC, LOADW], F16, kind="ExternalInput").ap()
    w1cat = dt("w1cat", [RVEC, HID], F16, kind="ExternalInput").ap()
    b1c = dt("b1c", [HID, 1], F32, kind="ExternalInput").ap()
    w2 = dt("w2", [HID, HID], F16, kind="ExternalInput").ap()
    b2c = dt("b2c", [HID, 1], F32, kind="ExternalInput").ap()
    w3 = dt("w3", [HID, NOUT], F16, kind="ExternalInput").ap()
    # FWL output: edge t*TILE + c*128 + p -> outp[t//GRP, p, (t%GRP)*OUTW + 4c + l]
    outp = dt("outp", [NGRP, 128, GRP * OUTW], F16, kind="ExternalOutput").ap()

    silu = mybir.ActivationFunctionType.Silu

    with tile.TileContext(nc) as tc:
        with tc.tile_pool(name="consts", bufs=1) as cpool:
            w1_sb = cpool.tile([RVEC, HID], F16)
            nc.sync.dma_start(out=w1_sb[:], in_=w1cat[:])
            b1_sb = cpool.tile([HID, 1], F32)
            nc.sync.dma_start(out=b1_sb[:], in_=b1c[:])
            w2_sb = cpool.tile([HID, HID], F16)
            nc.sync.dma_start(out=w2_sb[:], in_=w2[:])
            b2_sb = cpool.tile([HID, 1], F32)
            nc.sync.dma_start(out=b2_sb[:], in_=b2c[:])
            w3_sb = cpool.tile([HID, NOUT], F16)
            nc.sync.dma_start(out=w3_sb[:], in_=w3[:])

            with (
                tc.tile_pool(name="sin", bufs=2) as spool,
                tc.tile_pool(name="hh", bufs=3) as hpool,
                tc.tile_pool(name="oo", bufs=2) as opool,
                tc.tile_pool(name="p1", bufs=2, space="PSUM") as p1pool,
                tc.tile_pool(name="p2", bufs=1, space="PSUM") as p2pool,
                tc.tile_pool(name="p3", bufs=2, space="PSUM") as p3pool,
            ):
                sin_tiles = {}
                state = {"o_sb": None}

                def load(li):
                    s = spool.tile([RVEC, LOADW], F16, tag="sin")
                    nc.sync.dma_start(out=s[:], in_=stream[li])
                    sin_tiles[li] = s

                def head(t):
                    li, lo = divmod(t, TILES_PER_LOAD)
                    s = sin_tiles[li]
                    p1 = p1pool.tile([HID, TILE], F32, tag="p1")
                    for r in range(0, TILE, MMR):
                        c = lo * TILE + r
                        nc.tensor.matmul(out=p1[:, r:r + MMR], lhsT=w1_sb[:],
                                         rhs=s[:, c:c + MMR],
                                         start=True, stop=True)
                    h1 = hpool.tile([HID, TILE], F16, tag="h1")
                    nc.scalar.activation(out=h1[:], in_=p1[:], func=silu,
                                         bias=b1_sb[:])
                    return h1

                def mid(t, h1):
                    p2 = p2pool.tile([HID, TILE], F32, tag="p2")
                    for r in range(0, TILE, MMR):
                        nc.tensor.matmul(out=p2[:, r:r + MMR], lhsT=w2_sb[:],
                                         rhs=h1[:, r:r + MMR],
                                         start=True, stop=True)
                    h2 = hpool.tile([HID, TILE], F16, tag="h2")
                    nc.scalar.activation(out=h2[:], in_=p2[:], func=silu,
                                         bias=b2_sb[:])
                    return h2

                def tail(t, h2):
                    # final projection per 128-edge block: h2 block as
                    # stationary, logits land [128 edges, 4] (FWL).  The
                    # free-4 matmuls are overhead-only on PE, which beats
                    # two free-512 matmuls with W3 stationary (measured).
                    p3 = p3pool.tile([128, OUTW], F32, tag="p3")
                    for c in range(TILE // 128):
                        nc.tensor.matmul(out=p3[:, 4 * c:4 * c + 4],
                                         lhsT=h2[:, 128 * c:128 * c + 128],
                                         rhs=w3_sb[:],
                                         start=True, stop=True)
                    g, j = divmod(t, GRP)
                    if j == 0:
                        state["o_sb"] = opool.tile([128, GRP * OUTW], F16,
                                                   tag="o", name="o_sb")
                    o_sb = state["o_sb"]
                    nc.vector.tensor_copy(out=o_sb[:, j * OUTW:(j + 1) * OUTW],
                                          in_=p3[:])
                    if j == GRP - 1:
                        nc.gpsimd.dma_start(out=outp[g], in_=o_sb[:])

                # 2-stage software pipeline (head(t) | mid+tail(t-1)).
                # Deeper pipelining was measured slower: concurrent engines
                # contend (per-instruction latencies inflate ~20%).
                load(0)
                prev = None
                for t in range(NTILES):
                    if t % TILES_PER_LOAD == 0 and t // TILES_PER_LOAD + 1 < NLOADS:
                        load(t // TILES_PER_LOAD + 1)
                    h1 = head(t)
                    if prev is not None:
                        tail(t - 1, mid(t - 1, prev))
                    prev = h1
                tail(NTILES - 1, mid(NTILES - 1, prev))

    nc.compile()
    return nc


# ---------------------------------------------------------------- host side
def _prep(x, pos, edge_index, Wa, ba, Wp, bp, W1, b1, W2, b2, W3, b3):
    x = np.asarray(x, np.float32)
    pos = np.asarray(pos, np.float32)
    src = np.asarray(edge_index[0], np.int64)
    dst = np.asarray(edge_index[1], np.int64)
    E = src.shape[0]

    W1 = np.asarray(W1, np.float64)
    Wa64 = np.asarray(Wa, np.float64)
    Wp64 = np.asarray(Wp, np.float64)
    W1a = W1[:HID]
    W1b = W1[HID:2 * HID]
    w1c = W1[2 * HID:2 * HID + 1]                       # [1, 128]
    bab = (np.asarray(ba, np.float64) + np.asarray(bp, np.float64))
    w1cat = np.concatenate([
        Wa64 @ W1a, Wp64 @ W1a, Wa64 @ W1b, Wp64 @ W1b, w1c,
    ], axis=0).astype(np.float16)                       # [39, 128]
    b1f = (bab @ (W1a + W1b) + np.asarray(b1, np.float64)).astype(np.float32)
    b1c_ = np.ascontiguousarray(b1f[:, None])
    w2 = np.asarray(W2, np.float32).astype(np.float16)
    b2c = np.ascontiguousarray(np.asarray(b2, np.float32)[:, None])
    w3 = np.asarray(W3, np.float32).astype(np.float16)  # [128, 4]

    # dense per-edge feature stream, fp16
    feat = np.empty((E, RVEC), np.float16)
    feat[:, 0:ATOM] = x[src]
    feat[:, ATOM:ATOM + POSD] = pos[src]
    feat[:, ATOM + POSD:2 * ATOM + POSD] = x[dst]
    feat[:, 2 * ATOM + POSD:2 * (ATOM + POSD)] = pos[dst]
    diff = pos[src] - pos[dst]
    feat[:, RVEC - 1] = np.sqrt((diff * diff).sum(1))

    in_maps = []
    for c in range(N_CORES):
        S = np.zeros((ESLOT, RVEC), np.float16)
        S[:EC] = feat[c * EC:(c + 1) * EC]
        S = np.ascontiguousarray(
            S.reshape(NLOADS, LOADW, RVEC).transpose(0, 2, 1))
        in_maps.append({
            "stream": S, "w1cat": w1cat, "b1c": b1c_,
            "w2": w2, "b2c": b2c, "w3": w3,
        })
    return in_maps, np.asarray(b3, np.float32), E


def kernel(**inputs):
    if "prog" not in _CACHE:
        _CACHE["prog"] = _build_program()
    nc = _CACHE["prog"]

    in_maps, b3, E = _prep(**inputs)
    res = bass_utils.run_bass_kernel_spmd(nc, in_maps,
                                          core_ids=list(range(N_CORES)))

    out = np.empty((E, NOUT), np.float32)
    for c in range(N_CORES):
        o = np.asarray(res.results[c]["outp"])      # [NGRP, 128, GRP*OUTW]
        o = (o.reshape(NGRP, 128, GRP, TILE // 128, NOUT)
             .transpose(0, 2, 3, 1, 4).reshape(NTILES * TILE, NOUT))
        out[c * EC:(c + 1) * EC] = o[:EC]
    out += b3[None, :]
    return out


# revision 37
# speedup vs baseline: 1.1018x; 1.1018x over previous
"""BondPredictor (GNN message passing) Trainium2 kernel — 8 NeuronCores.

Strategy
--------
reference:
    node_emb = (x @ Wa + ba) + (pos @ Wp + bp)            # [N,128]
    e = concat([node_emb[src], node_emb[dst], dist], -1)  # [E,257]
    h = silu(e @ W1 + b1); h = silu(h @ W2 + b2); out = h @ W3 + b3

Key algebraic fold: because node_emb is linear in [x, pos, 1],

    e @ W1 + b1 = [x_s, p_s, x_d, p_d, dist] @ W1cat + b1'

with W1cat = [Wa@W1a; Wp@W1a; Wa@W1b; Wp@W1b; w1c]  (39 x 128) and
b1' = (ba+bp)@(W1a+W1b) + b1.  So the device kernel needs NO gathers at
all: the host lays out a dense per-edge feature stream of 39 fp16 rows
(src features, dst features, distance) in edge order, and each core runs
a pure 3-matmul MLP pipeline over its contiguous slice of edges:

    MM1 [39->128] -> silu -> MM2 [128->128] -> silu -> MM3 [128->4]

(A previous revision gathered 256B node_emb rows per edge with
dma_gather; the NTFF trace showed the Pool engine spending 3.5ms/core
generating DMA descriptors at ~8ns each — per-edge descriptors are the
real bottleneck on TRN2, so they were eliminated entirely.)

Sharding: contiguous edge ranges, 200000 edges per core.  b3 is added on
the host during output reassembly (it is a constant [4] vector).
"""

import sys

for _p in ("/opt/trn_rl_repo",):
    if _p not in sys.path:
        sys.path.insert(0, _p)

import numpy as np

import concourse.bass as bass
import concourse.bacc as bacc
import concourse.mybir as mybir
import concourse.tile as tile
from concourse import bass_utils

F16 = mybir.dt.float16
F32 = mybir.dt.float32

# ---------------------------------------------------------------- config
N_NODES = 100000
N_EDGES = 1600000
ATOM = 16
POSD = 3
HID = 128
NOUT = 4
N_CORES = 8

RVEC = 2 * (ATOM + POSD) + 1    # 39 stream rows per edge
EC = N_EDGES // N_CORES         # 200000 edges per core
TILE = 1024                     # edges per compute tile
MMR = 512                       # matmul free-dim region (one PSUM bank)
TILES_PER_LOAD = 4              # tiles per input DMA
LOADW = TILE * TILES_PER_LOAD   # 4096 edges per input DMA
NTILES = (EC + TILE - 1) // TILE            # 196
NLOADS = (NTILES + TILES_PER_LOAD - 1) // TILES_PER_LOAD  # 49
ESLOT = NLOADS * LOADW          # 200704 padded edge slots per core
OUTW = (TILE // 128) * NOUT     # 32 output cols per tile (FWL layout)
GRP = 7                         # tiles staged per output DMA
NGRP = NTILES // GRP            # 28

_CACHE = {}


# ---------------------------------------------------------------- program
def _build_program():
    nc = bacc.Bacc("TRN2", target_bir_lowering=False, debug=False,
                   num_devices=N_CORES)

    dt = nc.dram_tensor
    stream = dt("stream", [NLOADS, RVEC, LOADW], F16, kind="ExternalInput").ap()
    w1cat = dt("w1cat", [RVEC, HID], F16, kind="ExternalInput").ap()
    b1c = dt("b1c", [HID, 1], F32, kind="ExternalInput").ap()
    w2 = dt("w2", [HID, HID], F16, kind="ExternalInput").ap()
    b2c = dt("b2c", [HID, 1], F32, kind="ExternalInput").ap()
    # hidden-state output: h2 for edge t*TILE + e is houtp[t, :, e]; the
    # tiny final projection [128 -> 4] plus b3 runs on the host (BLAS sgemm)
    houtp = dt("houtp", [NTILES, HID, TILE], F16, kind="ExternalOutput").ap()

    silu = mybir.ActivationFunctionType.Silu

    with tile.TileContext(nc) as tc:
        with tc.tile_pool(name="consts", bufs=1) as cpool:
            w1_sb = cpool.tile([RVEC, HID], F16)
            nc.sync.dma_start(out=w1_sb[:], in_=w1cat[:])
            b1_sb = cpool.tile([HID, 1], F32)
            nc.sync.dma_start(out=b1_sb[:], in_=b1c[:])
            w2_sb = cpool.tile([HID, HID], F16)
            nc.sync.dma_start(out=w2_sb[:], in_=w2[:])
            b2_sb = cpool.tile([HID, 1], F32)
            nc.sync.dma_start(out=b2_sb[:], in_=b2c[:])

            with (
                tc.tile_pool(name="sin", bufs=2) as spool,
                tc.tile_pool(name="hh", bufs=3) as hpool,
                tc.tile_pool(name="p1", bufs=2, space="PSUM") as p1pool,
                tc.tile_pool(name="p2", bufs=1, space="PSUM") as p2pool,
            ):
                sin_tiles = {}

                def load(li):
                    s = spool.tile([RVEC, LOADW], F16, tag="sin")
                    nc.sync.dma_start(out=s[:], in_=stream[li])
                    sin_tiles[li] = s

                def head(t):
                    li, lo = divmod(t, TILES_PER_LOAD)
                    s = sin_tiles[li]
                    p1 = p1pool.tile([HID, TILE], F32, tag="p1")
                    for r in range(0, TILE, MMR):
                        c = lo * TILE + r
                        nc.tensor.matmul(out=p1[:, r:r + MMR], lhsT=w1_sb[:],
                                         rhs=s[:, c:c + MMR],
                                         start=True, stop=True)
                    h1 = hpool.tile([HID, TILE], F16, tag="h1")
                    nc.scalar.activation(out=h1[:], in_=p1[:], func=silu,
                                         bias=b1_sb[:])
                    return h1

                def mid(t, h1):
                    p2 = p2pool.tile([HID, TILE], F32, tag="p2")
                    for r in range(0, TILE, MMR):
                        nc.tensor.matmul(out=p2[:, r:r + MMR], lhsT=w2_sb[:],
                                         rhs=h1[:, r:r + MMR],
                                         start=True, stop=True)
                    h2 = hpool.tile([HID, TILE], F16, tag="h2")
                    nc.scalar.activation(out=h2[:], in_=p2[:], func=silu,
                                         bias=b2_sb[:])
                    # h2 leaves on the otherwise-idle Pool DMA queue; the
                    # final [128 -> 4] projection happens on the host
                    nc.gpsimd.dma_start(out=houtp[t], in_=h2[:])

                # 2-stage software pipeline (head(t) | mid(t-1)); deeper
                # pipelining was measured slower (engine contention inflates
                # per-instruction latencies ~20%)
                load(0)
                prev = None
                for t in range(NTILES):
                    if t % TILES_PER_LOAD == 0 and t // TILES_PER_LOAD + 1 < NLOADS:
                        load(t // TILES_PER_LOAD + 1)
                    h1 = head(t)
                    if prev is not None:
                        mid(t - 1, prev)
                    prev = h1
                mid(NTILES - 1, prev)

    nc.compile()
    return nc


# ---------------------------------------------------------------- host side
def _prep(x, pos, edge_index, Wa, ba, Wp, bp, W1, b1, W2, b2, W3, b3):
    x = np.asarray(x, np.float32)
    pos = np.asarray(pos, np.float32)
    src = np.asarray(edge_index[0], np.int64)
    dst = np.asarray(edge_index[1], np.int64)
    E = src.shape[0]

    W1 = np.asarray(W1, np.float64)
    Wa64 = np.asarray(Wa, np.float64)
    Wp64 = np.asarray(Wp, np.float64)
    W1a = W1[:HID]
    W1b = W1[HID:2 * HID]
    w1c = W1[2 * HID:2 * HID + 1]                       # [1, 128]
    bab = (np.asarray(ba, np.float64) + np.asarray(bp, np.float64))
    w1cat = np.concatenate([
        Wa64 @ W1a, Wp64 @ W1a, Wa64 @ W1b, Wp64 @ W1b, w1c,
    ], axis=0).astype(np.float16)                       # [39, 128]
    b1f = (bab @ (W1a + W1b) + np.asarray(b1, np.float64)).astype(np.float32)
    b1c_ = np.ascontiguousarray(b1f[:, None])
    w2 = np.asarray(W2, np.float32).astype(np.float16)
    b2c = np.ascontiguousarray(np.asarray(b2, np.float32)[:, None])
    w3 = np.asarray(W3, np.float32)                     # [128, 4], host-side

    # dense per-edge feature stream, fp16
    feat = np.empty((E, RVEC), np.float16)
    feat[:, 0:ATOM] = x[src]
    feat[:, ATOM:ATOM + POSD] = pos[src]
    feat[:, ATOM + POSD:2 * ATOM + POSD] = x[dst]
    feat[:, 2 * ATOM + POSD:2 * (ATOM + POSD)] = pos[dst]
    diff = pos[src] - pos[dst]
    feat[:, RVEC - 1] = np.sqrt((diff * diff).sum(1))

    in_maps = []
    for c in range(N_CORES):
        S = np.zeros((ESLOT, RVEC), np.float16)
        S[:EC] = feat[c * EC:(c + 1) * EC]
        S = np.ascontiguousarray(
            S.reshape(NLOADS, LOADW, RVEC).transpose(0, 2, 1))
        in_maps.append({
            "stream": S, "w1cat": w1cat, "b1c": b1c_,
            "w2": w2, "b2c": b2c,
        })
    return in_maps, w3, np.asarray(b3, np.float32), E


def kernel(**inputs):
    if "prog" not in _CACHE:
        _CACHE["prog"] = _build_program()
    nc = _CACHE["prog"]

    in_maps, w3, b3, E = _prep(**inputs)
    res = bass_utils.run_bass_kernel_spmd(nc, in_maps,
                                          core_ids=list(range(N_CORES)))

    out = np.empty((E, NOUT), np.float32)

    def _finish(c):
        o = np.asarray(res.results[c]["houtp"])     # [NTILES, 128, TILE]
        H = (o[: (EC + TILE - 1) // TILE].transpose(0, 2, 1)
             .reshape(-1, HID)[:EC].astype(np.float32))
        out[c * EC:(c + 1) * EC] = H @ w3 + b3[None, :]

    from concurrent.futures import ThreadPoolExecutor
    with ThreadPoolExecutor(max_workers=N_CORES) as ex:
        list(ex.map(_finish, range(N_CORES)))
    return out
